# revision 7
# baseline (speedup 1.0000x reference)
"""GCN autoencoder (2-layer GCN + inner-product decoder) on 8 Trainium2 NeuronCores.

Strategy (dest-row sharding, dense-A formulation):
  - A (10000x10000, 320k nnz) is materialized dense in bf16 on the host and
    pre-tiled per core as A^T k-tiles so each core streams 25MB/layer at full
    DMA rate. PE contracts over source rows (K=128 tiles).
  - 4 SPMD launches; the host only reshapes/concatenates shards in between
    (the "all-gather" of the tiny XW1 / t / z tables):
      L1: XW1 shard  = x_shard @ W1                       (per-core rows)
      L2: h1^T tiles = relu(H1^T-contracted A tiles); t^T = W2^T @ h1^T
      L3: z^T shard  = A @ (h1 W2) contracted the same way
      L4: out shard  = z_shard @ z^T   (50MB/core write -> memory roofline)
  - Everything except A/H-table streaming is f32; PSUM accumulation is f32.
"""

import sys

sys.path.insert(0, "/opt/trn_rl_repo")

import numpy as np
import ml_dtypes

import concourse.bacc as bacc
import concourse.mybir as mybir
import concourse.tile as tile
from concourse.bass_utils import run_bass_kernel_spmd

BF16 = ml_dtypes.bfloat16

NC = 8
N = 10000
F = 512
H1 = 32
H2 = 16
M_SH = N // NC            # 1250 rows per core
M_PAD = 1280              # padded to 10 x 128
KT = 79                   # source-dim k-tiles (79*128 = 10112 >= 10000)
SRC_PAD = KT * 128
KX = F // 128             # 4 k-tiles for the x @ W1 matmul
A_BATCH = 4               # A k-tiles per DMA (4 * 320KB = 1.28MB)

N_SLICES_PAD = [(0, 512), (512, 512), (1024, 256)]    # 1280 cols
M_TILES = [(i * 128, min(128, M_SH - i * 128)) for i in range((M_SH + 127) // 128)]
N_SLICES = [(0, 512), (512, 512), (1024, 226)]          # 1250 cols
DEC_N = [(i * 512, min(512, N - i * 512)) for i in range((N + 511) // 512)]

_cache = {}
_last_maps = {}


def _mt(*shape):
    return list(shape)


def _new_nc():
    return bacc.Bacc("TRN2", target_bir_lowering=False, debug=False, num_devices=NC)


# ---------------------------------------------------------------- L1: x @ W1
def _build_l1():
    nc = _new_nc()
    xT = nc.dram_tensor("xT", _mt(128, KX, M_PAD), mybir.dt.float32, kind="ExternalInput")
    w1 = nc.dram_tensor("w1", _mt(128, KX, H1), mybir.dt.float32, kind="ExternalInput")
    out = nc.dram_tensor("xw1T", _mt(H1, M_PAD), mybir.dt.float32, kind="ExternalOutput")
    with tile.TileContext(nc) as tc:
        with (
            tc.tile_pool(name="sb", bufs=1) as sb,
            tc.tile_pool(name="ps", bufs=4, space="PSUM") as ps,
        ):
            xt = sb.tile([128, KX, M_PAD], mybir.dt.float32)
            w1t = sb.tile([128, KX, H1], mybir.dt.float32)
            stage = sb.tile([H1, M_PAD], mybir.dt.float32)
            nc.sync.dma_start(out=xt[:], in_=xT[:])
            nc.sync.dma_start(out=w1t[:], in_=w1[:])
            for ni, (n0, nn) in enumerate(N_SLICES_PAD):
                acc = ps.tile([H1, 512], mybir.dt.float32, tag="acc")
                for k in range(KX):
                    nc.tensor.matmul(
                        out=acc[:, :nn],
                        lhsT=w1t[:, k, :],
                        rhs=xt[:, k, n0:n0 + nn],
                        start=(k == 0),
                        stop=(k == KX - 1),
                    )
                nc.vector.tensor_copy(out=stage[:, n0:n0 + nn], in_=acc[:, :nn])
            nc.sync.dma_start(out=out[:], in_=stage[:])
    nc.compile()
    return nc


# ------------------------------------------------- L2/L3: A-contraction layer
def _build_gcn_layer(h_dim, relu_w2):
    """out^T[f, m] = sum_s H[s, f] * A[m_dest, s]  (+ optional relu + @W2).

    Inputs:
      at  [KT, 128, M_PAD]  bf16   A^T tiles (partition = source row)
      h   [128, KT, h_dim]  bf16   wrapped source table
      w2  [32, 16] f32             (only when relu_w2)
    Output: [16 or h_dim, M_PAD] f32  (t^T shard or z^T shard)
    """
    nc = _new_nc()
    at = nc.dram_tensor("at", _mt(128, KT, M_PAD), mybir.dt.bfloat16, kind="ExternalInput")
    h = nc.dram_tensor("h", _mt(128, KT, h_dim), mybir.dt.bfloat16, kind="ExternalInput")
    if relu_w2:
        w2 = nc.dram_tensor("w2", _mt(H1, H2), mybir.dt.float32, kind="ExternalInput")
    out_rows = H2 if relu_w2 else h_dim
    out = nc.dram_tensor("outT", _mt(out_rows, M_PAD), mybir.dt.float32, kind="ExternalOutput")

    n_bat = (KT + A_BATCH - 1) // A_BATCH
    with tile.TileContext(nc) as tc:
        with (
            tc.tile_pool(name="sb", bufs=1) as sb,
            tc.tile_pool(name="ab", bufs=3) as ab,
            tc.tile_pool(name="ps", bufs=1, space="PSUM") as ps,
            tc.tile_pool(name="ps2", bufs=2, space="PSUM") as ps2,
        ):
            hsb = sb.tile([128, KT, h_dim], mybir.dt.bfloat16)
            nc.sync.dma_start(out=hsb[:], in_=h[:])
            if relu_w2:
                w2sb = sb.tile([H1, H2], mybir.dt.float32)
                nc.sync.dma_start(out=w2sb[:], in_=w2[:])
            accs = [ps.tile([h_dim, nn], mybir.dt.float32, tag=f"acc{ni}", name=f"acc{ni}")
                    for ni, (n0, nn) in enumerate(N_SLICES)]
            for b in range(n_bat):
                k0 = b * A_BATCH
                nk = min(A_BATCH, KT - k0)
                a_t = ab.tile([128, nk, M_PAD], mybir.dt.bfloat16, tag="a")
                nc.sync.dma_start(out=a_t[:], in_=at[:, k0:k0 + nk, :])
                for kk in range(nk):
                    kt = k0 + kk
                    for ni, (n0, nn) in enumerate(N_SLICES):
                        nc.tensor.matmul(
                            out=accs[ni][:],
                            lhsT=hsb[:, kt, :],
                            rhs=a_t[:, kk, n0:n0 + nn],
                            start=(kt == 0),
                            stop=(kt == KT - 1),
                        )
            if relu_w2:
                h1T = sb.tile([H1, M_SH], mybir.dt.float32)
                for ni, (n0, nn) in enumerate(N_SLICES):
                    nc.scalar.activation(
                        out=h1T[:, n0:n0 + nn],
                        in_=accs[ni][:],
                        func=mybir.ActivationFunctionType.Relu,
                    )
                stage = sb.tile([H2, M_PAD], mybir.dt.float32)
                nc.vector.memset(stage[:], 0.0)
                for ni, (n0, nn) in enumerate(N_SLICES):
                    tacc = ps2.tile([H2, nn], mybir.dt.float32, tag="tacc")
                    nc.tensor.matmul(
                        out=tacc[:], lhsT=w2sb[:], rhs=h1T[:, n0:n0 + nn],
                        start=True, stop=True,
                    )
                    nc.vector.tensor_copy(out=stage[:, n0:n0 + nn], in_=tacc[:])
                nc.sync.dma_start(out=out[:], in_=stage[:])
            else:
                stage = sb.tile([h_dim, M_PAD], mybir.dt.float32)
                nc.vector.memset(stage[:], 0.0)
                for ni, (n0, nn) in enumerate(N_SLICES):
                    nc.vector.tensor_copy(out=stage[:, n0:n0 + nn], in_=accs[ni][:])
                nc.sync.dma_start(out=out[:], in_=stage[:])
    nc.compile()
    return nc


# ---------------------------------------------------------------- L4: decode
def _build_l4():
    nc = _new_nc()
    zt = nc.dram_tensor("zt", _mt(H2, N), mybir.dt.float32, kind="ExternalInput")
    ztl = nc.dram_tensor("ztl", _mt(H2, M_PAD), mybir.dt.float32, kind="ExternalInput")
    out = nc.dram_tensor("out", _mt(M_SH, N), mybir.dt.float32, kind="ExternalOutput")
    with tile.TileContext(nc) as tc:
        with (
            tc.tile_pool(name="sb", bufs=1) as sb,
            tc.tile_pool(name="stg", bufs=2) as stg,
            tc.tile_pool(name="ps", bufs=8, space="PSUM") as ps,
        ):
            ztsb = sb.tile([H2, N], mybir.dt.float32)
            ztlsb = sb.tile([H2, M_PAD], mybir.dt.float32)
            nc.sync.dma_start(out=ztsb[:], in_=zt[:])
            nc.sync.dma_start(out=ztlsb[:], in_=ztl[:])
            ztr = sb.tile([H2, N], mybir.dt.float32r)
            ztlr = sb.tile([H2, M_PAD], mybir.dt.float32r)
            nc.vector.tensor_copy(out=ztlr[:], in_=ztlsb[:])
            for n0, nn in DEC_N:
                nc.vector.tensor_copy(out=ztr[:, n0:n0 + nn], in_=ztsb[:, n0:n0 + nn])
            for mi, (m0, mm) in enumerate(M_TILES):
                stage = stg.tile([128, N], mybir.dt.float32, tag="stage")
                for ni, (n0, nn) in enumerate(DEC_N):
                    acc = ps.tile([128, 512], mybir.dt.float32, tag="acc")
                    nc.tensor.matmul(
                        out=acc[:mm, :nn],
                        lhsT=ztlr[:, m0:m0 + mm],
                        rhs=ztr[:, n0:n0 + nn],
                        start=True, stop=True,
                    )
                    nc.vector.tensor_copy(out=stage[:mm, n0:n0 + nn], in_=acc[:mm, :nn])
                nc.sync.dma_start(out=out[m0:m0 + mm, :], in_=stage[:mm, :])
    nc.compile()
    return nc


def _get(name, builder):
    if name not in _cache:
        _cache[name] = builder()
    return _cache[name]


def _run(nc, in_maps, name=None):
    if name is not None:
        _last_maps[name] = in_maps
    return run_bass_kernel_spmd(nc, in_maps, list(range(NC))).results


def kernel(x, edge_w, W1, W2, edge_row, edge_col):
    x = np.asarray(x, np.float32)
    edge_w = np.asarray(edge_w, np.float32)
    W1 = np.asarray(W1, np.float32)
    W2 = np.asarray(W2, np.float32)
    er = np.asarray(edge_row).astype(np.int64)
    ec = np.asarray(edge_col).astype(np.int64)

    # dense A^T, padded: atp[s, m] = A[m, s]
    atp = np.zeros((SRC_PAD, N), np.float32)
    np.add.at(atp, (ec, er), edge_w)  # atp[src, dest] += w
    atp_bf = atp.astype(BF16)

    # per-core A^T tiles: [KT, 128, M_PAD]
    at_tiles = []
    for c in range(NC):
        blk = np.zeros((128, KT, M_PAD), BF16)
        blk[:, :, :M_SH] = atp_bf[:, c * M_SH:(c + 1) * M_SH].reshape(KT, 128, M_SH).transpose(1, 0, 2)
        at_tiles.append(blk)

    # ---- L1: XW1 shards
    xT = x.T  # [512, 10000]
    l1 = _get("l1", _build_l1)
    w1w = W1.reshape(KX, 128, H1).transpose(1, 0, 2).copy()  # [128, KX, H1]
    maps = []
    for c in range(NC):
        xt_c = np.zeros((128, KX, M_PAD), np.float32)
        xs = xT[:, c * M_SH:(c + 1) * M_SH].reshape(KX, 128, M_SH).transpose(1, 0, 2)
        xt_c[:, :, :M_SH] = xs
        maps.append({"xT": xt_c, "w1": w1w})
    res = _run(l1, maps, "l1")
    # assemble H1 [SRC_PAD, 32] from transposed shard outputs [H1, M_PAD]
    xw1 = np.zeros((SRC_PAD, H1), np.float32)
    for c in range(NC):
        xw1[c * M_SH:(c + 1) * M_SH] = res[c]["xw1T"][:, :M_SH].T
    h1_wrapped = xw1.reshape(KT, 128, H1).transpose(1, 0, 2).astype(BF16).copy()

    # ---- L2: t^T shards
    l2 = _get("l2", lambda: _build_gcn_layer(H1, True))
    maps = [{"at": at_tiles[c], "h": h1_wrapped, "w2": W2} for c in range(NC)]
    res = _run(l2, maps, "l2")
    t_full = np.zeros((SRC_PAD, H2), np.float32)
    for c in range(NC):
        t_full[c * M_SH:(c + 1) * M_SH] = res[c]["outT"][:, :M_SH].T
    h2_wrapped = t_full.reshape(KT, 128, H2).transpose(1, 0, 2).astype(BF16).copy()

    # ---- L3: z^T shards
    l3 = _get("l3", lambda: _build_gcn_layer(H2, False))
    maps = [{"at": at_tiles[c], "h": h2_wrapped} for c in range(NC)]
    res = _run(l3, maps, "l3")
    zt_full = np.zeros((H2, N), np.float32)
    for c in range(NC):
        zt_full[:, c * M_SH:(c + 1) * M_SH] = res[c]["outT"][:, :M_SH]

    # ---- L4: decode
    l4 = _get("l4", _build_l4)
    maps = []
    for c in range(NC):
        ztl = np.zeros((H2, M_PAD), np.float32)
        ztl[:, :M_SH] = zt_full[:, c * M_SH:(c + 1) * M_SH]
        maps.append({"zt": zt_full, "ztl": ztl})
    res = _run(l4, maps, "l4")
    out = np.concatenate([res[c]["out"] for c in range(NC)], axis=0)
    return out.reshape(-1)


# revision 8
# speedup vs baseline: 1.0761x; 1.0761x over previous
"""GCN autoencoder (2-layer GCN + inner-product decoder) on 8 Trainium2 NeuronCores.

Strategy (dest-row sharding, dense-A formulation):
  - A (10000x10000, 320k nnz) is materialized dense in bf16 on the host and
    pre-tiled per core as A^T k-tiles so each core streams 25MB/layer at full
    DMA rate. PE contracts over source rows (K=128 tiles).
  - 4 SPMD launches; the host only reshapes/concatenates shards in between
    (the "all-gather" of the tiny XW1 / t / z tables):
      L1: XW1 shard  = x_shard @ W1                       (per-core rows)
      L2: h1^T tiles = relu(H1^T-contracted A tiles); t^T = W2^T @ h1^T
      L3: z^T shard  = A @ (h1 W2) contracted the same way
      L4: out shard  = z_shard @ z^T   (50MB/core write -> memory roofline)
  - Everything except A/H-table streaming is f32; PSUM accumulation is f32.
"""

import sys

sys.path.insert(0, "/opt/trn_rl_repo")

import numpy as np
import ml_dtypes

import concourse.bacc as bacc
import concourse.mybir as mybir
import concourse.tile as tile
from concourse.bass_utils import run_bass_kernel_spmd

BF16 = ml_dtypes.bfloat16

NC = 8
N = 10000
F = 512
H1 = 32
H2 = 16
M_SH = N // NC            # 1250 rows per core
M_PAD = 1280              # padded to 10 x 128
KT = 79                   # source-dim k-tiles (79*128 = 10112 >= 10000)
SRC_PAD = KT * 128
KX = F // 128             # 4 k-tiles for the x @ W1 matmul
A_BATCH = 8               # A k-tiles per DMA (4 * 320KB = 1.28MB)

N_SLICES_PAD = [(0, 512), (512, 512), (1024, 256)]    # 1280 cols
M_TILES = [(i * 128, min(128, M_SH - i * 128)) for i in range((M_SH + 127) // 128)]
N_SLICES = [(0, 512), (512, 512), (1024, 226)]          # 1250 cols
DEC_N = [(i * 512, min(512, N - i * 512)) for i in range((N + 511) // 512)]

_cache = {}
_last_maps = {}


def _mt(*shape):
    return list(shape)


def _new_nc():
    return bacc.Bacc("TRN2", target_bir_lowering=False, debug=False, num_devices=NC)


# ---------------------------------------------------------------- L1: x @ W1
def _build_l1():
    nc = _new_nc()
    xT = nc.dram_tensor("xT", _mt(128, KX, M_PAD), mybir.dt.float32, kind="ExternalInput")
    w1 = nc.dram_tensor("w1", _mt(128, KX, H1), mybir.dt.float32, kind="ExternalInput")
    out = nc.dram_tensor("xw1T", _mt(H1, M_PAD), mybir.dt.float32, kind="ExternalOutput")
    with tile.TileContext(nc) as tc:
        with (
            tc.tile_pool(name="sb", bufs=1) as sb,
            tc.tile_pool(name="ps", bufs=4, space="PSUM") as ps,
        ):
            xt = sb.tile([128, KX, M_PAD], mybir.dt.float32)
            w1t = sb.tile([128, KX, H1], mybir.dt.float32)
            stage = sb.tile([H1, M_PAD], mybir.dt.float32)
            nc.sync.dma_start(out=xt[:], in_=xT[:])
            nc.sync.dma_start(out=w1t[:], in_=w1[:])
            for ni, (n0, nn) in enumerate(N_SLICES_PAD):
                acc = ps.tile([H1, 512], mybir.dt.float32, tag="acc")
                for k in range(KX):
                    nc.tensor.matmul(
                        out=acc[:, :nn],
                        lhsT=w1t[:, k, :],
                        rhs=xt[:, k, n0:n0 + nn],
                        start=(k == 0),
                        stop=(k == KX - 1),
                    )
                nc.vector.tensor_copy(out=stage[:, n0:n0 + nn], in_=acc[:, :nn])
            nc.sync.dma_start(out=out[:], in_=stage[:])
    nc.compile()
    return nc


# ------------------------------------------------- L2/L3: A-contraction layer
def _build_gcn_layer(h_dim, relu_w2):
    """out^T[f, m] = sum_s H[s, f] * A[m_dest, s]  (+ optional relu + @W2).

    Inputs:
      at  [KT, 128, M_PAD]  bf16   A^T tiles (partition = source row)
      h   [128, KT, h_dim]  bf16   wrapped source table
      w2  [32, 16] f32             (only when relu_w2)
    Output: [16 or h_dim, M_PAD] f32  (t^T shard or z^T shard)
    """
    nc = _new_nc()
    at = nc.dram_tensor("at", _mt(128, KT, M_PAD), mybir.dt.bfloat16, kind="ExternalInput")
    h = nc.dram_tensor("h", _mt(128, KT, h_dim), mybir.dt.bfloat16, kind="ExternalInput")
    if relu_w2:
        w2 = nc.dram_tensor("w2", _mt(H1, H2), mybir.dt.float32, kind="ExternalInput")
    out_rows = H2 if relu_w2 else h_dim
    out = nc.dram_tensor("outT", _mt(out_rows, M_PAD), mybir.dt.float32, kind="ExternalOutput")

    n_bat = (KT + A_BATCH - 1) // A_BATCH
    with tile.TileContext(nc) as tc:
        with (
            tc.tile_pool(name="sb", bufs=1) as sb,
            tc.tile_pool(name="ab", bufs=3) as ab,
            tc.tile_pool(name="ps", bufs=1, space="PSUM") as ps,
            tc.tile_pool(name="ps2", bufs=2, space="PSUM") as ps2,
        ):
            hsb = sb.tile([128, KT, h_dim], mybir.dt.bfloat16)
            nc.sync.dma_start(out=hsb[:], in_=h[:])
            if relu_w2:
                w2sb = sb.tile([H1, H2], mybir.dt.float32)
                nc.sync.dma_start(out=w2sb[:], in_=w2[:])
            accs = [ps.tile([h_dim, nn], mybir.dt.float32, tag=f"acc{ni}", name=f"acc{ni}")
                    for ni, (n0, nn) in enumerate(N_SLICES)]
            for b in range(n_bat):
                k0 = b * A_BATCH
                nk = min(A_BATCH, KT - k0)
                a_t = ab.tile([128, nk, M_PAD], mybir.dt.bfloat16, tag="a")
                nc.sync.dma_start(out=a_t[:], in_=at[:, k0:k0 + nk, :])
                for kk in range(nk):
                    kt = k0 + kk
                    for ni, (n0, nn) in enumerate(N_SLICES):
                        nc.tensor.matmul(
                            out=accs[ni][:],
                            lhsT=hsb[:, kt, :],
                            rhs=a_t[:, kk, n0:n0 + nn],
                            start=(kt == 0),
                            stop=(kt == KT - 1),
                        )
            if relu_w2:
                h1T = sb.tile([H1, M_SH], mybir.dt.float32)
                for ni, (n0, nn) in enumerate(N_SLICES):
                    nc.scalar.activation(
                        out=h1T[:, n0:n0 + nn],
                        in_=accs[ni][:],
                        func=mybir.ActivationFunctionType.Relu,
                    )
                stage = sb.tile([H2, M_PAD], mybir.dt.float32)
                nc.vector.memset(stage[:], 0.0)
                for ni, (n0, nn) in enumerate(N_SLICES):
                    tacc = ps2.tile([H2, nn], mybir.dt.float32, tag="tacc")
                    nc.tensor.matmul(
                        out=tacc[:], lhsT=w2sb[:], rhs=h1T[:, n0:n0 + nn],
                        start=True, stop=True,
                    )
                    nc.vector.tensor_copy(out=stage[:, n0:n0 + nn], in_=tacc[:])
                nc.sync.dma_start(out=out[:], in_=stage[:])
            else:
                stage = sb.tile([h_dim, M_PAD], mybir.dt.float32)
                nc.vector.memset(stage[:], 0.0)
                for ni, (n0, nn) in enumerate(N_SLICES):
                    nc.vector.tensor_copy(out=stage[:, n0:n0 + nn], in_=accs[ni][:])
                nc.sync.dma_start(out=out[:], in_=stage[:])
    nc.compile()
    return nc


# ---------------------------------------------------------------- L4: decode
def _build_l4():
    nc = _new_nc()
    zt = nc.dram_tensor("zt", _mt(H2, N), mybir.dt.float32, kind="ExternalInput")
    ztl = nc.dram_tensor("ztl", _mt(H2, M_PAD), mybir.dt.float32, kind="ExternalInput")
    out = nc.dram_tensor("out", _mt(M_SH, N), mybir.dt.float32, kind="ExternalOutput")
    with tile.TileContext(nc) as tc:
        with (
            tc.tile_pool(name="sb", bufs=1) as sb,
            tc.tile_pool(name="stg", bufs=3) as stg,
            tc.tile_pool(name="ps", bufs=8, space="PSUM") as ps,
        ):
            ztsb = sb.tile([H2, N], mybir.dt.float32)
            ztlsb = sb.tile([H2, M_PAD], mybir.dt.float32)
            nc.sync.dma_start(out=ztsb[:], in_=zt[:])
            nc.sync.dma_start(out=ztlsb[:], in_=ztl[:])
            ztr = sb.tile([H2, N], mybir.dt.float32r)
            ztlr = sb.tile([H2, M_PAD], mybir.dt.float32r)
            nc.vector.tensor_copy(out=ztlr[:], in_=ztlsb[:])
            for n0, nn in DEC_N:
                nc.vector.tensor_copy(out=ztr[:, n0:n0 + nn], in_=ztsb[:, n0:n0 + nn])
            for mi, (m0, mm) in enumerate(M_TILES):
                stage = stg.tile([128, N], mybir.dt.float32, tag="stage")
                for ni, (n0, nn) in enumerate(DEC_N):
                    acc = ps.tile([128, 512], mybir.dt.float32, tag="acc")
                    nc.tensor.matmul(
                        out=acc[:mm, :nn],
                        lhsT=ztlr[:, m0:m0 + mm],
                        rhs=ztr[:, n0:n0 + nn],
                        start=True, stop=True,
                    )
                    nc.vector.tensor_copy(out=stage[:mm, n0:n0 + nn], in_=acc[:mm, :nn])
                nc.sync.dma_start(out=out[m0:m0 + mm, :N // 2], in_=stage[:mm, :N // 2])
                nc.sync.dma_start(out=out[m0:m0 + mm, N // 2:], in_=stage[:mm, N // 2:])
    nc.compile()
    return nc


def _get(name, builder):
    if name not in _cache:
        _cache[name] = builder()
    return _cache[name]


def _run(nc, in_maps, name=None):
    if name is not None:
        _last_maps[name] = in_maps
    return run_bass_kernel_spmd(nc, in_maps, list(range(NC))).results


def kernel(x, edge_w, W1, W2, edge_row, edge_col):
    x = np.asarray(x, np.float32)
    edge_w = np.asarray(edge_w, np.float32)
    W1 = np.asarray(W1, np.float32)
    W2 = np.asarray(W2, np.float32)
    er = np.asarray(edge_row).astype(np.int64)
    ec = np.asarray(edge_col).astype(np.int64)

    # dense A^T, padded: atp[s, m] = A[m, s]
    atp = np.zeros((SRC_PAD, N), np.float32)
    np.add.at(atp, (ec, er), edge_w)  # atp[src, dest] += w
    atp_bf = atp.astype(BF16)

    # per-core A^T tiles: [KT, 128, M_PAD]
    at_tiles = []
    for c in range(NC):
        blk = np.zeros((128, KT, M_PAD), BF16)
        blk[:, :, :M_SH] = atp_bf[:, c * M_SH:(c + 1) * M_SH].reshape(KT, 128, M_SH).transpose(1, 0, 2)
        at_tiles.append(blk)

    # ---- L1: XW1 shards
    xT = x.T  # [512, 10000]
    l1 = _get("l1", _build_l1)
    w1w = W1.reshape(KX, 128, H1).transpose(1, 0, 2).copy()  # [128, KX, H1]
    maps = []
    for c in range(NC):
        xt_c = np.zeros((128, KX, M_PAD), np.float32)
        xs = xT[:, c * M_SH:(c + 1) * M_SH].reshape(KX, 128, M_SH).transpose(1, 0, 2)
        xt_c[:, :, :M_SH] = xs
        maps.append({"xT": xt_c, "w1": w1w})
    res = _run(l1, maps, "l1")
    # assemble H1 [SRC_PAD, 32] from transposed shard outputs [H1, M_PAD]
    xw1 = np.zeros((SRC_PAD, H1), np.float32)
    for c in range(NC):
        xw1[c * M_SH:(c + 1) * M_SH] = res[c]["xw1T"][:, :M_SH].T
    h1_wrapped = xw1.reshape(KT, 128, H1).transpose(1, 0, 2).astype(BF16).copy()

    # ---- L2: t^T shards
    l2 = _get("l2", lambda: _build_gcn_layer(H1, True))
    maps = [{"at": at_tiles[c], "h": h1_wrapped, "w2": W2} for c in range(NC)]
    res = _run(l2, maps, "l2")
    t_full = np.zeros((SRC_PAD, H2), np.float32)
    for c in range(NC):
        t_full[c * M_SH:(c + 1) * M_SH] = res[c]["outT"][:, :M_SH].T
    h2_wrapped = t_full.reshape(KT, 128, H2).transpose(1, 0, 2).astype(BF16).copy()

    # ---- L3: z^T shards
    l3 = _get("l3", lambda: _build_gcn_layer(H2, False))
    maps = [{"at": at_tiles[c], "h": h2_wrapped} for c in range(NC)]
    res = _run(l3, maps, "l3")
    zt_full = np.zeros((H2, N), np.float32)
    for c in range(NC):
        zt_full[:, c * M_SH:(c + 1) * M_SH] = res[c]["outT"][:, :M_SH]

    # ---- L4: decode
    l4 = _get("l4", _build_l4)
    maps = []
    for c in range(NC):
        ztl = np.zeros((H2, M_PAD), np.float32)
        ztl[:, :M_SH] = zt_full[:, c * M_SH:(c + 1) * M_SH]
        maps.append({"zt": zt_full, "ztl": ztl})
    res = _run(l4, maps, "l4")
    out = np.concatenate([res[c]["out"] for c in range(NC)], axis=0)
    return out.reshape(-1)
